# revision 29
# baseline (speedup 1.0000x reference)
"""DARTS recurrent cell kernel for Trainium2 (8 NeuronCores, data-parallel over batch).

Strategy
--------
* Host side: fuse the "low-rank" params into dense matrices
  (W = U @ diag(sigma) @ V, exploiting that the zero-padding in the
  reference means only the first NHID rows of Ws_V contribute), cast to
  bf16, and lay out weights/inputs in the feature-major block layout the
  device kernel wants.
* Device side (per core, batch slice of 32): T=128 fully-unrolled
  sequential steps. Activation-stationary matmuls: the state s^T [K=128
  features, 32 batch] is the PE stationary operand, bf16 weights stream as
  the moving operand, accumulating [32, 1700] fp32 preactivations in PSUM.
  ScalarE casts PSUM->bf16, the DMA xbar transposes [32, 1792] ->
  [128, 14, 32] (feature-major, f = 128*chunk + partition), gates/state
  update run full-width on ScalarE/VectorE in fp32 (fp32 state carry ->
  ~3e-5 relative error), and hiddens exit each step via a bf16 hi/lo
  double-transpose that reconstructs exact fp32 batch-major rows.
* 6 of the 8 node weight matrices stay SBUF-resident; W0(x-part),
  W0(h-part) and the last two node matrices stream from HBM every step,
  overlapped with compute.
"""

import sys

sys.path.insert(0, "/opt/trn_rl_repo")

from contextlib import ExitStack

import ml_dtypes
import numpy as np

import concourse.bass as bass
import concourse.tile as tile
from concourse import bacc, mybir

# ---------------------------------------------------------------- constants
NINP = 850
NHID = 850
T = 128
B = 256
NCORES = 8
BC = B // NCORES          # batch per core = 32
H = NHID                  # 850
HP = 896                  # padded features = 7 * 128
KT = 7                    # K tiles of 128
F = 2 * NHID              # 1700 output features per matmul
FP = 1792                 # padded (2 * 896) for the transposed view
N_CHUNKS = [(0, 512), (512, 1024), (1024, 1536), (1536, 1700)]

GENO = [("sigmoid", 0), ("relu", 1), ("relu", 1), ("identity", 1),
        ("tanh", 2), ("sigmoid", 5), ("tanh", 3), ("relu", 5)]

N_RESIDENT = 6            # node weight matrices kept in SBUF

BF16 = mybir.dt.bfloat16
F32 = mybir.dt.float32
AFT = mybir.ActivationFunctionType
ACT_FN = {"sigmoid": AFT.Sigmoid, "tanh": AFT.Tanh, "relu": AFT.Relu}

NP_BF16 = ml_dtypes.bfloat16


# ---------------------------------------------------------------- host prep
def _pad_rows_to_tiles(w):
    """[rows<=896, N] f32 -> [128, 7, N] with row r stored at [r % 128, r // 128]."""
    out = np.zeros((HP, w.shape[1]), np.float32)
    out[: w.shape[0]] = w
    return np.ascontiguousarray(out.reshape(KT, 128, w.shape[1]).transpose(1, 0, 2))


def _prep_weights(W0_U, W0_sigma, W0_V, Ws_U, Ws_sigma, Ws_V):
    W0 = (W0_U * W0_sigma[None, :]) @ W0_V                       # [1700, 1700]
    w0x = _pad_rows_to_tiles(W0[:NHID]).astype(NP_BF16)          # [128, 7, 1700]
    w0h = _pad_rows_to_tiles(W0[NHID:]).astype(NP_BF16)          # [128, 7, 1700]
    wn = np.stack([
        _pad_rows_to_tiles((Ws_U[i] * Ws_sigma[i][None, :]) @ Ws_V[i][:NHID])
        for i in range(8)
    ]).astype(NP_BF16)                                           # [8, 128, 7, 1700]
    return w0x, w0h, wn


def _to_feature_major(x):
    """[T?, b, 850] -> [T?, 128, 7, b]: feature f = 128*m + p at [p, m]."""
    lead = x.shape[:-2]
    b = x.shape[-2]
    out = np.zeros(lead + (b, HP), np.float32)
    out[..., :H] = x
    out = out.reshape(lead + (b, KT, 128))
    # [..., b, m, p] -> [..., p, m, b]
    return np.ascontiguousarray(np.moveaxis(out, (-3, -2, -1), (-1, -2, -3)))


def feature_mask():
    """[128, 7, 32] bf16: 1 where feature 128*m + p < 850, else 0."""
    f = 128 * np.arange(KT)[None, :] + np.arange(128)[:, None]   # [128, 7]
    return np.broadcast_to((f < H)[:, :, None], (128, KT, BC)).astype(NP_BF16).copy()


# ---------------------------------------------------------------- device kernel
def build_kernel(n_steps=T):
    nc = bacc.Bacc("TRN2", target_bir_lowering=False, debug=False)

    x_d = nc.dram_tensor("x", [n_steps, 128, KT, BC], BF16, kind="ExternalInput").ap()
    w0x_d = nc.dram_tensor("w0x", [128, KT, F], BF16, kind="ExternalInput").ap()
    w0h_d = nc.dram_tensor("w0h", [128, KT, F], BF16, kind="ExternalInput").ap()
    wn_d = nc.dram_tensor("wn", [8, 128, KT, F], BF16, kind="ExternalInput").ap()
    h0f_d = nc.dram_tensor("h0f", [128, KT, BC], F32, kind="ExternalInput").ap()
    h0b_d = nc.dram_tensor("h0b", [128, KT, BC], BF16, kind="ExternalInput").ap()
    fmask_d = nc.dram_tensor("fmask", [128, KT, BC], BF16, kind="ExternalInput").ap()
    hid_d = nc.dram_tensor("hid", [n_steps, BC, H], F32, kind="ExternalOutput").ap()

    with tile.TileContext(nc) as tc, ExitStack() as ctx:
        wres = ctx.enter_context(tc.tile_pool(name="wres", bufs=1))
        wstr = ctx.enter_context(tc.tile_pool(name="wstr", bufs=4))
        xs = ctx.enter_context(tc.tile_pool(name="xs", bufs=3))
        sf = ctx.enter_context(tc.tile_pool(name="sf", bufs=13))
        sb = ctx.enter_context(tc.tile_pool(name="sb", bufs=12))
        gt = ctx.enter_context(tc.tile_pool(name="gt", bufs=2))
        gm = ctx.enter_context(tc.tile_pool(name="gm", bufs=1))
        chbp = ctx.enter_context(tc.tile_pool(name="chbp", bufs=2))
        chtp = ctx.enter_context(tc.tile_pool(name="chtp", bufs=3))
        outp = ctx.enter_context(tc.tile_pool(name="outp", bufs=1))
        ones = ctx.enter_context(tc.tile_pool(name="ones", bufs=1))
        psum = ctx.enter_context(tc.tile_pool(name="psum", bufs=2, space="PSUM"))

        # ---- resident node weights -------------------------------------
        wn_sb = []
        for i in range(N_RESIDENT):
            w = wres.tile([128, KT, F], BF16, tag=f"wn{i}")
            nc.sync.dma_start(out=w, in_=wn_d[i])
            wn_sb.append(w)

        # ---- initial hidden state --------------------------------------
        h_f = wres.tile([128, KT, BC], F32, tag="h0f")
        nc.sync.dma_start(out=h_f, in_=h0f_d)
        h_b = wres.tile([128, KT, BC], BF16, tag="h0b")
        nc.sync.dma_start(out=h_b, in_=h0b_d)
        fmask = wres.tile([128, KT, BC], BF16, tag="fmask")
        nc.sync.dma_start(out=fmask, in_=fmask_d)

        # persistent hi/lo staging for the output transpose. The xbar reads
        # the input free index as q = 128*m + b (fixed 128-wide tiles), so the
        # staging layout is [128, 8, 128] with batch in the first 32 of each
        # 128 block. Unused slots are zeroed once and never rewritten.
        hi_t = ones.tile([128, 8, 128], BF16, tag="hi")
        lo_t = ones.tile([128, 8, 128], BF16, tag="lo")
        nc.vector.memset(hi_t, 0.0)
        nc.vector.memset(lo_t, 0.0)

        def stream_weight(dram_mat, k, tag="wst"):
            w = wstr.tile([128, F], BF16, tag=tag)
            nc.sync.dma_start(out=w, in_=dram_mat[:, k, :])
            return w

        def mm_group(ch, parts):
            """parts: list of (lhsT [128, BC] bf16, rhs_tile [128, F]).

            k-outer / chunk-inner so each streamed weight tile's slot frees
            after 4 consecutive matmuls (keeps the wstr pool small).
            """
            for k, (lhsT, rhs) in enumerate(parts):
                for n0, n1 in N_CHUNKS:
                    nc.tensor.matmul(
                        ch[:, n0:n1], lhsT=lhsT, rhs=rhs[:, n0:n1],
                        start=(k == 0), stop=(k == len(parts) - 1),
                        skip_group_check=True,
                    )

        def gates(ch, act_name, sp_f):
            """sigmoid/act gates + state update; returns (s_f32, s_bf16)."""
            # cols [850:896) / [1746:1792) are junk fill (finite values from the
            # neighboring psum region); they land in pad feature slots that are
            # zeroed in s_f below and never reach the output.
            chb = chbp.tile([32, FP], BF16, tag="chb")
            nc.scalar.copy(chb[:, 0:HP], ch[:, 0:HP])
            nc.scalar.copy(chb[:, HP:HP + H], ch[:, H:F])
            nc.scalar.copy(chb[:, HP + H:FP], ch[:, 0:HP - H])
            cht = chtp.tile([128, 2 * KT, BC], BF16, tag="cht")
            nc.sync.dma_start_transpose(out=cht, in_=chb)

            c_f = gt.tile([128, KT, BC], F32, tag="gc")
            nc.scalar.activation(c_f, cht[:, 0:KT, :], AFT.Sigmoid)
            if act_name == "identity":
                hh = cht[:, KT:2 * KT, :]
            else:
                hh = gt.tile([128, KT, BC], F32, tag="gh")
                nc.scalar.activation(hh, cht[:, KT:2 * KT, :], ACT_FN[act_name])

            d = gt.tile([128, KT, BC], F32, tag="gd")
            nc.vector.tensor_sub(d, hh, sp_f)
            e = gt.tile([128, KT, BC], F32, tag="ge")
            nc.vector.tensor_mul(e, c_f, d)
            s_f = sf.tile([128, KT, BC], F32, tag="sf")
            nc.vector.tensor_add(s_f, sp_f, e)
            # masked bf16 cast: zeroes the padded feature slots (f >= 850) so
            # no garbage can reach the matmul stationary operand.
            s_b = sb.tile([128, KT, BC], BF16, tag="sb")
            nc.vector.tensor_mul(s_b, s_f, fmask)
            return s_f, s_b

        for t in range(n_steps):
            xt = xs.tile([128, KT, BC], BF16, tag="x")
            nc.sync.dma_start(out=xt, in_=x_d[t])

            w0x_t = [stream_weight(w0x_d, k) for k in range(KT)]
            w0h_t = [stream_weight(w0h_d, k) for k in range(KT)]
            # stream the non-resident node weights at k-tile granularity
            wn_stream = {
                i: [stream_weight(wn_d[i], k) for k in range(KT)]
                for i in range(N_RESIDENT, 8)
            }

            # ---- s0: ch0 = [x_t; h] @ W0 --------------------------------
            ch = psum.tile([32, F], F32, tag="ch")
            parts = [(xt[:, k, :], w0x_t[k]) for k in range(KT)]
            parts += [(h_b[:, k, :], w0h_t[k]) for k in range(KT)]
            mm_group(ch, parts)
            states = [gates(ch, "tanh", h_f)]

            # ---- nodes ---------------------------------------------------
            for i, (act_name, pred) in enumerate(GENO):
                sp_f, sp_b = states[pred]
                ch = psum.tile([32, F], F32, tag="ch")
                if i < N_RESIDENT:
                    parts = [(sp_b[:, k, :], wn_sb[i][:, k, :]) for k in range(KT)]
                else:
                    parts = [(sp_b[:, k, :], wn_stream[i][k]) for k in range(KT)]
                mm_group(ch, parts)
                states.append(gates(ch, act_name, sp_f))

            # ---- h = mean(s1..s8) ---------------------------------------
            a = [states[j][0] for j in range(1, 9)]
            m01 = gm.tile([128, KT, BC], F32, tag="m0")
            nc.vector.tensor_add(m01, a[0], a[1])
            m23 = gm.tile([128, KT, BC], F32, tag="m1")
            nc.vector.tensor_add(m23, a[2], a[3])
            m45 = gm.tile([128, KT, BC], F32, tag="m2")
            nc.vector.tensor_add(m45, a[4], a[5])
            m67 = gm.tile([128, KT, BC], F32, tag="m3")
            nc.vector.tensor_add(m67, a[6], a[7])
            nc.vector.tensor_add(m01, m01, m23)
            nc.vector.tensor_add(m45, m45, m67)
            msum = gm.tile([128, KT, BC], F32, tag="m4")
            nc.vector.tensor_add(msum, m01, m45)
            h_f = sf.tile([128, KT, BC], F32, tag="sf")
            nc.scalar.mul(h_f, msum, 0.125)
            h_b = sb.tile([128, KT, BC], BF16, tag="sb")
            nc.vector.tensor_mul(h_b, h_f, fmask)

            # ---- emit hiddens[t] (exact fp32 via bf16 hi/lo pair) -------
            nc.gpsimd.tensor_copy(hi_t[:, 0:KT, 0:BC], h_f)
            nc.vector.tensor_sub(lo_t[:, 0:KT, 0:BC], h_f, hi_t[:, 0:KT, 0:BC])
            hiT = outp.tile([32, 8, 128], BF16, tag="hiT")
            nc.sync.dma_start_transpose(out=hiT, in_=hi_t)
            loT = outp.tile([32, 8, 128], BF16, tag="loT")
            nc.sync.dma_start_transpose(out=loT, in_=lo_t)
            hsum = outp.tile([32, 8 * 128], F32, tag="hsum")
            nc.vector.tensor_add(
                hsum, hiT.rearrange("b m p -> b (m p)"),
                loT.rearrange("b m p -> b (m p)"))
            nc.sync.dma_start(out=hid_d[t], in_=hsum[:, 0:H])

    nc.compile()
    return nc


# ---------------------------------------------------------------- entry point
def kernel(inputs, hidden, W0_U, W0_sigma, W0_V, Ws_U, Ws_sigma, Ws_V):
    from concourse.bass_utils import run_bass_kernel_spmd

    inputs = np.asarray(inputs, np.float32)
    hidden = np.asarray(hidden, np.float32)
    w0x, w0h, wn = _prep_weights(
        np.asarray(W0_U, np.float32), np.asarray(W0_sigma, np.float32),
        np.asarray(W0_V, np.float32), np.asarray(Ws_U, np.float32),
        np.asarray(Ws_sigma, np.float32), np.asarray(Ws_V, np.float32))

    nc = build_kernel(T)
    fmask = feature_mask()
    in_maps = []
    for c in range(NCORES):
        sl = slice(c * BC, (c + 1) * BC)
        xc = _to_feature_major(inputs[:, sl, :])          # [T, 128, 7, 32] f32
        h0 = _to_feature_major(hidden[0, sl, :])          # [128, 7, 32] f32
        in_maps.append({
            "x": xc.astype(NP_BF16),
            "w0x": w0x, "w0h": w0h, "wn": wn,
            "h0f": h0, "h0b": h0.astype(NP_BF16), "fmask": fmask,
        })

    res = run_bass_kernel_spmd(nc, in_maps, core_ids=list(range(NCORES)))
    hiddens = np.empty((T, B, NHID), np.float32)
    for c in range(NCORES):
        hiddens[:, c * BC:(c + 1) * BC, :] = res.results[c]["hid"]
    return hiddens, hiddens[-1][None]
